# revision 19
# baseline (speedup 1.0000x reference)
"""BiAttn kernel for 8 TRN2 NeuronCores.

Math: the additive score e[b,x,y] = (k[b,x]@Wk) + (q[b,y]@Wq) + b is constant
along y up to the sq term, and softmax is shift-invariant, so
    a[b,x,y] = softmax(sq[b,:])[y]   (independent of x)
    out[b,x,h] = sum_y p[b,y] * v[b,y,h] = c[b,h]   for every x.
k and the scalar bias cancel entirely. Per batch: sq = q@Wq, p = exp(sq)/sum,
c = p@v, out = broadcast(c) over X. sq ~ N(0, 0.5) so exp() without
max-subtraction is safe.

Sharding: batch B=8 -> one batch per core, fully data parallel, no
collectives. Per core: read q_b, v_b (16MB f32), write out_b.

Implementation notes:
- Inputs are cast f32->bf16 during the SWDGE (gpsimd) input DMA; all
  on-chip math except the reduce/psum accumulation runs bf16 (fp32
  matmuls on TRN2 run two LOW_HIGH passes; bf16 is single-pass).
  Measured end-to-end rel err ~5e-3 vs the 2e-2 gate.
- Engine split: DVE does the q*Wq multiply (one op per chunk, Wq
  broadcast via a stride-0 AP); the free-dim reduction rides the Scalar
  engine as activation(Copy, accum_out=...) so DVE stays off the
  critical path; PE accumulates c (and d, via a ones column) in PSUM
  with esq columns as the stationary operand; softmax exp is ACT.
- All input chunks are fully buffered (bufs=len(CHUNKS)) so the input
  stream free-runs at HBM rate; the last chunks are 1 tile so the
  post-last-DMA tail is short. The 1/d scale is folded into the ones
  vector of the K=1 broadcast matmul.
- Measured ~71-75us per NEFF on a quiet chip (input stream ~16.5MB
  read HBM-bound, output 4MB bf16 write, ~14us NEFF fixed overhead).
"""

import sys

import numpy as np

for _p in ("/opt/trn_rl_repo",):
    if _p not in sys.path:
        sys.path.insert(0, _p)

B, X, Y, H = 8, 2048, 2048, 1024
N_CORES = 8
P = 128            # partitions
NT = Y // P        # 16 y-tiles
CHUNKS = [2, 2, 2, 2, 2, 2, 2, 1, 1]   # y-tiles per input DMA chunk
assert sum(CHUNKS) == NT

OUT_DTYPE = "bfloat16"  # output DRAM dtype; host upcasts to f32
SINGLE_OUT_DMA = False  # one broadcast-source dma_start vs NT plain ones

_cache = {}


def _build():
    import concourse.bass as bass
    import concourse.mybir as mybir
    from concourse import bacc, tile

    f32 = mybir.dt.float32
    bf16 = mybir.dt.bfloat16
    out_dt = getattr(mybir.dt, OUT_DTYPE)

    nc = bacc.Bacc("TRN2", target_bir_lowering=False, debug=False,
                   num_devices=N_CORES, name="biattn")

    q = nc.dram_tensor("q", [Y, H], f32, kind="ExternalInput").ap()
    v = nc.dram_tensor("v", [Y, H], f32, kind="ExternalInput").ap()
    wq = nc.dram_tensor("wq", [P, H], f32, kind="ExternalInput").ap()
    out = nc.dram_tensor("out", [X, H], out_dt, kind="ExternalOutput").ap()

    # per-tile view: tile yt covers rows [yt*128, (yt+1)*128)
    q_t = q.rearrange("(n p) h -> n p h", p=P)
    v_t = v.rearrange("(n p) h -> n p h", p=P)

    with tile.TileContext(nc) as tc:
        with (
            tc.tile_pool(name="const", bufs=1) as constp,
            tc.tile_pool(name="qin", bufs=len(CHUNKS)) as qp,
            tc.tile_pool(name="vin", bufs=len(CHUNKS)) as vp,
            tc.tile_pool(name="scr", bufs=3) as scr,
            tc.tile_pool(name="small", bufs=1) as smallp,
            tc.tile_pool(name="ps_acc", bufs=1, space=bass.MemorySpace.PSUM) as psa,
            tc.tile_pool(name="ps_misc", bufs=2, space=bass.MemorySpace.PSUM) as psm,
        ):
            wq_b = constp.tile([P, H], bf16, tag="wq_b", name="wq_b")
            nc.gpsimd.dma_start(wq_b[:], wq)

            ones_row = constp.tile([1, P], bf16, tag="ones_row", name="ones_row")
            nc.vector.memset(ones_row[:], 1.0)
            ones_col = constp.tile([P, 1], bf16, tag="ones_col", name="ones_col")
            nc.vector.memset(ones_col[:], 1.0)

            sq_all = smallp.tile([P, NT], f32, tag="sq_all", name="sq_all")
            esq_all = smallp.tile([P, NT], bf16, tag="esq_all", name="esq_all")

            ps_c = psa.tile([1, H], f32, tag="ps_c", name="ps_c")
            ps_d = psa.tile([1, 1], f32, tag="ps_d", name="ps_d")

            yt = 0
            for ci, cs in enumerate(CHUNKS):
                q_sb = qp.tile([P, cs * H], bf16, tag="q_sb", name="q_sb",
                               padded_shape=[P, max(CHUNKS) * H])
                nc.gpsimd.dma_start(
                    q_sb[:].rearrange("p (t h) -> p t h", t=cs),
                    q_t[yt:yt + cs].rearrange("n p h -> p n h"))
                v_bf = vp.tile([P, cs * H], bf16, tag="v_bf", name="v_bf",
                               padded_shape=[P, max(CHUNKS) * H])
                nc.gpsimd.dma_start(
                    v_bf[:].rearrange("p (t h) -> p t h", t=cs),
                    v_t[yt:yt + cs].rearrange("n p h -> p n h"))

                sc = scr.tile([P, cs * H], bf16, tag="sc", name="sc",
                              padded_shape=[P, max(CHUNKS) * H])
                nc.vector.tensor_mul(
                    sc[:].rearrange("p (t h) -> p t h", t=cs),
                    q_sb[:].rearrange("p (t h) -> p t h", t=cs),
                    wq_b[:].unsqueeze(1).broadcast_to([P, cs, H]))
                for t in range(cs):
                    if yt == NT - 1:
                        nc.vector.reduce_sum(
                            sq_all[:, yt:yt + 1], sc[:, t * H:(t + 1) * H],
                            axis=mybir.AxisListType.X)
                    else:
                        dump = scr.tile([P, H], bf16, tag="dump", name="dump")
                        nc.scalar.activation(
                            dump[:], sc[:, t * H:(t + 1) * H],
                            mybir.ActivationFunctionType.Copy,
                            accum_out=sq_all[:, yt:yt + 1])
                    nc.scalar.activation(
                        esq_all[:, yt:yt + 1], sq_all[:, yt:yt + 1],
                        mybir.ActivationFunctionType.Exp)
                    if yt == NT - 1:
                        # d first so reciprocal overlaps the last c-matmuls
                        mms = [("d", None), ("c", 0), ("c", 1)]
                    else:
                        mms = [("c", 0), ("c", 1), ("d", None)]
                    for kind, j in mms:
                        if kind == "c":
                            nc.tensor.matmul(
                                ps_c[:, j * 512:(j + 1) * 512],
                                esq_all[:, yt:yt + 1],
                                v_bf[:, t * H + j * 512:t * H + (j + 1) * 512],
                                start=(yt == 0), stop=(yt == NT - 1))
                        else:
                            nc.tensor.matmul(
                                ps_d[:], esq_all[:, yt:yt + 1], ones_col[:],
                                start=(yt == 0), stop=(yt == NT - 1))
                    yt += 1

            # c = psum_c / d; fold 1/d into the broadcast matmul's ones
            inv_d = smallp.tile([1, 1], f32, tag="inv_d", name="inv_d")
            nc.vector.reciprocal(inv_d[:], ps_d[:])
            ones_sc = smallp.tile([1, P], bf16, tag="ones_sc", name="ones_sc")
            nc.vector.tensor_scalar_mul(ones_sc[:], ones_row[:], inv_d[:])
            c_sb = smallp.tile([1, H], bf16, tag="c_sb", name="c_sb")
            nc.vector.tensor_copy(c_sb[:, 0:512], ps_c[:, 0:512])
            nc.vector.tensor_copy(c_sb[:, 512:H], ps_c[:, 512:H])

            # broadcast c/d to all 128 partitions via K=1 matmul
            bc_sb = smallp.tile([P, H], out_dt, tag="bc_sb", name="bc_sb")
            for j in range(2):
                ps_b = psm.tile([P, 512], f32, tag="ps_b", name="ps_b")
                nc.tensor.matmul(ps_b[:], ones_sc[:],
                                 c_sb[:, j * 512:(j + 1) * 512],
                                 start=True, stop=True)
                if j == 0:
                    nc.vector.tensor_copy(bc_sb[:, 0:512], ps_b[:])
                else:
                    nc.scalar.copy(bc_sb[:, 512:H], ps_b[:])

            if SINGLE_OUT_DMA:
                dest = out.rearrange("(t p) h -> p t h", p=P)
                src = bc_sb[:].unsqueeze(1).broadcast_to([P, NT, H])
                nc.sync.dma_start(dest, src)
            else:
                out_r = out.rearrange("(t p) h -> t p h", p=P)
                for t in range(NT):
                    eng = nc.sync if t % 2 == 0 else nc.scalar
                    eng.dma_start(out_r[t], bc_sb[:])
    nc.compile()
    return nc


def _get_nc():
    if "nc" not in _cache:
        _cache["nc"] = _build()
    return _cache["nc"]


def _in_maps(q, k, v, W, b):
    q = np.asarray(q, dtype=np.float32)
    v = np.asarray(v, dtype=np.float32)
    W = np.asarray(W, dtype=np.float32)
    wq = np.ascontiguousarray(np.broadcast_to(W[H:], (P, H)))
    return [
        {"q": np.ascontiguousarray(q[c]),
         "v": np.ascontiguousarray(v[c]),
         "wq": wq}
        for c in range(N_CORES)
    ]


def kernel(q, k, v, W, b):
    from concourse.bass_utils import run_bass_kernel_spmd

    nc = _get_nc()
    res = run_bass_kernel_spmd(nc, _in_maps(q, k, v, W, b),
                               core_ids=list(range(N_CORES)))
    outs = [np.asarray(res.results[c]["out"]).astype(np.float32)
            for c in range(N_CORES)]
    return np.stack(outs)


# revision 22
# speedup vs baseline: 1.0903x; 1.0903x over previous
"""BiAttn kernel for 8 TRN2 NeuronCores.

Math: the additive score e[b,x,y] = (k[b,x]@Wk) + (q[b,y]@Wq) + b is constant
along y up to the sq term, and softmax is shift-invariant, so
    a[b,x,y] = softmax(sq[b,:])[y]   (independent of x)
    out[b,x,h] = sum_y p[b,y] * v[b,y,h] = c[b,h]   for every x.
k and the scalar bias cancel entirely. Per batch: sq = q@Wq, p = exp(sq)/sum,
c = p@v, out = broadcast(c) over X. sq ~ N(0, 0.5) so exp() without
max-subtraction is safe.

Sharding: batch B=8 -> one batch per core, fully data parallel, no
collectives. Per core: read q_b, v_b (16MB f32), write out_b (4MB bf16,
upcast to f32 on host). Measured rel err ~3e-3 vs the 2e-2 gate.

Pipeline (per core), designed so the DMA engines never idle:
1. q streams first (SWDGE DMAs casting f32->bf16 inline). As chunks land:
   DVE multiplies by Wq (stride-0 broadcast AP), the row-reduction
   alternates between ScalarE activation(Copy, accum_out=) and DVE
   reduce_sum, exp on ScalarE, and DVE replicates each exp(sq) column
   into a [128,128] stationary tile esq_b.
2. PE accumulates d = sum(esq) via esq_b @ ones into a [128,1] PSUM (every
   partition gets d), so c and 1/d are "pre-broadcast" and no epilogue
   broadcast matmul is needed.
3. v streams in column halves (all y-tiles of h[0:512], then h[512:1024]).
   PE accumulates c_half = esq_b @ v_half into [128,512] PSUM. When half 0
   finishes, its scaled output (c0/d) is written to DRAM *while half 1 is
   still streaming in* - the 2MB write hides under the 4MB read.
4. Tail after the last v packet is just one matmul + scale-copy + DMA.

Notes: bf16 everywhere on the engines (fp32 matmuls cost two LOW_HIGH
passes); f32 only for sq/esq scalars and PSUM accumulation. Output DMAs
alternate between the two HWDGE rings (sync/scalar). All input chunks are
fully buffered so the stream free-runs at HBM rate (~380-420GB/s/core).
Measured 67.4-75us per NEFF (fleet-noise dependent; ~14us is fixed NEFF
entry/exit overhead)."""

import sys

import numpy as np

for _p in ("/opt/trn_rl_repo",):
    if _p not in sys.path:
        sys.path.insert(0, _p)

B, X, Y, H = 8, 2048, 2048, 1024
N_CORES = 8
P = 128
NT = Y // P
CHUNKS = [2, 2, 2, 2, 2, 2, 2, 1, 1]
assert sum(CHUNKS) == NT
OUT_DTYPE = "bfloat16"

_cache = {}


def _build():
    import concourse.bass as bass
    import concourse.mybir as mybir
    from concourse import bacc, tile

    f32 = mybir.dt.float32
    bf16 = mybir.dt.bfloat16
    out_dt = getattr(mybir.dt, OUT_DTYPE)

    nc = bacc.Bacc("TRN2", target_bir_lowering=False, debug=False,
                   num_devices=N_CORES, name="biattn")

    q = nc.dram_tensor("q", [Y, H], f32, kind="ExternalInput").ap()
    v = nc.dram_tensor("v", [Y, H], f32, kind="ExternalInput").ap()
    wq = nc.dram_tensor("wq", [P, H], f32, kind="ExternalInput").ap()
    out = nc.dram_tensor("out", [X, H], out_dt, kind="ExternalOutput").ap()

    q_t = q.rearrange("(n p) h -> n p h", p=P)
    v_t = v.rearrange("(n p) h -> n p h", p=P)
    out_r = out.rearrange("(t p) h -> t p h", p=P)

    with tile.TileContext(nc) as tc:
        with (
            tc.tile_pool(name="const", bufs=1) as constp,
            tc.tile_pool(name="qin", bufs=len(CHUNKS)) as qp,
            tc.tile_pool(name="vin", bufs=2 * len(CHUNKS)) as vp,
            tc.tile_pool(name="scr", bufs=3) as scr,
            tc.tile_pool(name="ebp", bufs=NT) as ebp,
            tc.tile_pool(name="small", bufs=1) as smallp,
            tc.tile_pool(name="ps_acc", bufs=1, space=bass.MemorySpace.PSUM) as psa,
        ):
            wq_b = constp.tile([P, H], bf16, tag="wq_b", name="wq_b")
            nc.gpsimd.dma_start(wq_b[:], wq)

            ones_col = constp.tile([P, 1], bf16, tag="ones_col", name="ones_col")
            nc.vector.memset(ones_col[:], 1.0)
            ones_big = constp.tile([P, P], bf16, tag="ones_big", name="ones_big")
            nc.vector.memset(ones_big[:], 1.0)

            sq_all = smallp.tile([P, NT], f32, tag="sq_all", name="sq_all")
            esq_all = smallp.tile([P, NT], f32, tag="esq_all", name="esq_all")

            ps_c0 = psa.tile([P, 512], f32, tag="ps_c0", name="ps_c0")
            ps_c1 = psa.tile([P, 512], f32, tag="ps_c1", name="ps_c1")
            ps_d = psa.tile([P, 1], f32, tag="ps_d", name="ps_d")

            starts = [sum(CHUNKS[:i]) for i in range(len(CHUNKS))]
            q_tiles = [qp.tile([P, cs * H], bf16, tag="q_sb",
                               name=f"q_sb{i}",
                               padded_shape=[P, max(CHUNKS) * H])
                       for i, cs in enumerate(CHUNKS)]
            # v half-column tiles: [P, cs*512] per (chunk, half)
            v_tiles = [[vp.tile([P, cs * 512], bf16, tag="v_bf",
                                name=f"v_bf{i}_{j}",
                                padded_shape=[P, max(CHUNKS) * 512])
                        for j in range(2)]
                       for i, cs in enumerate(CHUNKS)]

            # ---- DMA issue order: q stream, then v half 0, then v half 1
            for i, cs in enumerate(CHUNKS):
                s = starts[i]
                nc.gpsimd.dma_start(
                    q_tiles[i][:].rearrange("p (t h) -> p t h", t=cs),
                    q_t[s:s + cs].rearrange("n p h -> p n h"))
            for j in range(2):
                for i, cs in enumerate(CHUNKS):
                    s = starts[i]
                    src = v_t[s:s + cs, :, j * 512:(j + 1) * 512]
                    nc.gpsimd.dma_start(
                        v_tiles[i][j][:].rearrange("p (t h) -> p t h", t=cs),
                        src.rearrange("n p h -> p n h"))

            # ---- sq / esq / esq_b / d, paced with the q stream
            esq_bs = []
            yt = 0
            for ci, cs in enumerate(CHUNKS):
                q_sb = q_tiles[ci]
                sc = scr.tile([P, cs * H], bf16, tag="sc", name="sc",
                              padded_shape=[P, max(CHUNKS) * H])
                nc.vector.tensor_mul(
                    sc[:].rearrange("p (t h) -> p t h", t=cs),
                    q_sb[:].rearrange("p (t h) -> p t h", t=cs),
                    wq_b[:].unsqueeze(1).broadcast_to([P, cs, H]))
                for t in range(cs):
                    if yt % 2 == 1:
                        nc.vector.reduce_sum(
                            sq_all[:, yt:yt + 1], sc[:, t * H:(t + 1) * H],
                            axis=mybir.AxisListType.X)
                    else:
                        dump = scr.tile([P, H], bf16, tag="dump", name="dump")
                        nc.scalar.activation(
                            dump[:], sc[:, t * H:(t + 1) * H],
                            mybir.ActivationFunctionType.Copy,
                            accum_out=sq_all[:, yt:yt + 1])
                    nc.scalar.activation(
                        esq_all[:, yt:yt + 1], sq_all[:, yt:yt + 1],
                        mybir.ActivationFunctionType.Exp)
                    esq_b = ebp.tile([P, P], bf16, tag="esq_b",
                                     name=f"esq_b{yt}")
                    nc.vector.tensor_scalar_mul(
                        esq_b[:], ones_big[:], esq_all[:, yt:yt + 1])
                    esq_bs.append(esq_b)
                    nc.tensor.matmul(
                        ps_d[:], esq_b[:], ones_col[:],
                        start=(yt == 0), stop=(yt == NT - 1))
                    yt += 1

            inv_d = smallp.tile([P, 1], f32, tag="inv_d", name="inv_d")
            nc.vector.reciprocal(inv_d[:], ps_d[:])

            bc_sb = smallp.tile([P, H], out_dt, tag="bc_sb", name="bc_sb")

            # ---- per-half: accumulate c_j over all tiles, scale, write out
            for j in range(2):
                ps = [ps_c0, ps_c1][j]
                yt = 0
                for ci, cs in enumerate(CHUNKS):
                    for t in range(cs):
                        nc.tensor.matmul(
                            ps[:], esq_bs[yt],
                            v_tiles[ci][j][:, t * 512:(t + 1) * 512],
                            start=(yt == 0), stop=(yt == NT - 1))
                        yt += 1
                if j == 0:
                    nc.vector.tensor_scalar_mul(
                        bc_sb[:, 0:512], ps[:], inv_d[:])
                else:
                    nc.scalar.activation(
                        bc_sb[:, 512:H], ps[:],
                        mybir.ActivationFunctionType.Copy, scale=inv_d[:])
                for t in range(NT):
                    eng = nc.sync if t % 2 == 0 else nc.scalar
                    eng.dma_start(out_r[t, :, j * 512:(j + 1) * 512],
                                  bc_sb[:, j * 512:(j + 1) * 512])
    nc.compile()
    return nc


def _get_nc():
    if "nc" not in _cache:
        _cache["nc"] = _build()
    return _cache["nc"]


def _in_maps(q, k, v, W, b):
    q = np.asarray(q, dtype=np.float32)
    v = np.asarray(v, dtype=np.float32)
    W = np.asarray(W, dtype=np.float32)
    wq = np.ascontiguousarray(np.broadcast_to(W[H:], (P, H)))
    return [
        {"q": np.ascontiguousarray(q[c]),
         "v": np.ascontiguousarray(v[c]),
         "wq": wq}
        for c in range(N_CORES)
    ]


def kernel(q, k, v, W, b):
    from concourse.bass_utils import run_bass_kernel_spmd

    nc = _get_nc()
    res = run_bass_kernel_spmd(nc, _in_maps(q, k, v, W, b),
                               core_ids=list(range(N_CORES)))
    outs = [np.asarray(res.results[c]["out"]).astype(np.float32)
            for c in range(N_CORES)]
    return np.stack(outs)


# revision 23
# speedup vs baseline: 1.0927x; 1.0022x over previous
"""BiAttn kernel for 8 TRN2 NeuronCores.

Math: the additive score e[b,x,y] = (k[b,x]@Wk) + (q[b,y]@Wq) + b is constant
along y up to the sq term, and softmax is shift-invariant, so
    a[b,x,y] = softmax(sq[b,:])[y]   (independent of x)
    out[b,x,h] = sum_y p[b,y] * v[b,y,h] = c[b,h]   for every x.
k and the scalar bias cancel entirely. Per batch: sq = q@Wq, p = exp(sq)/sum,
c = p@v, out = broadcast(c) over X. sq ~ N(0, 0.5) so exp() without
max-subtraction is safe.

Sharding: batch B=8 -> one batch per core, fully data parallel, no
collectives. Per core: read q_b, v_b (16MB f32), write out_b (4MB bf16,
upcast to f32 on host). Measured rel err ~3e-3 vs the 2e-2 gate.

Pipeline (per core), built so the DMA engines never idle and the first
half of the output write overlaps the tail of the input stream:
1. q streams first (SWDGE DMAs casting f32->bf16 inline), then v in
   column halves (all y-tiles of h[0:512], then h[512:1024]).
2. As q chunks land: DVE multiplies by Wq (stride-0 broadcast AP); the
   row-reduction alternates ScalarE activation(Copy, accum_out=) / DVE
   reduce_sum; exp on ScalarE; DVE replicates exp(sq) into a [128,128]
   stationary tile esq_b.
3. PE work is interleaved per tile (d-matmul, then c0-matmul) so the
   c0 accumulation tracks the vh0 stream instead of queuing behind the
   last d-matmul in the PE FIFO. d = esq_b @ ones gives the softmax
   denominator already broadcast on all partitions; c0/c1 = esq_b @
   v_half give c broadcast too, so no epilogue broadcast matmul exists.
4. When c0 closes, ACT scales it by 1/d and its 2MB output writes while
   vh1 still streams in; c1 accumulates behind the vh1 stream, DVE
   scales it (ACT sequencer is busy issuing h0 DMAs), and only the
   final 2MB write plus one matmul+scale is serial tail.

Notes: bf16 on all engines (fp32 matmuls cost two LOW_HIGH passes); f32
only for sq/esq scalars and PSUM accumulation. Output DMAs alternate
between the two HWDGE rings (sync/scalar). All input chunks are fully
buffered so the stream free-runs at HBM rate. Measured 65.8-78us per
NEFF depending on fleet noise (~14us is fixed NEFF entry/exit)."""

import sys

import numpy as np

for _p in ("/opt/trn_rl_repo",):
    if _p not in sys.path:
        sys.path.insert(0, _p)

B, X, Y, H = 8, 2048, 2048, 1024
N_CORES = 8
P = 128
NT = Y // P
CHUNKS = [2, 2, 2, 2, 2, 2, 2, 1, 1]
assert sum(CHUNKS) == NT
OUT_DTYPE = "bfloat16"

_cache = {}


def _build():
    import concourse.bass as bass
    import concourse.mybir as mybir
    from concourse import bacc, tile

    f32 = mybir.dt.float32
    bf16 = mybir.dt.bfloat16
    out_dt = getattr(mybir.dt, OUT_DTYPE)

    nc = bacc.Bacc("TRN2", target_bir_lowering=False, debug=False,
                   num_devices=N_CORES, name="biattn")

    q = nc.dram_tensor("q", [Y, H], f32, kind="ExternalInput").ap()
    v = nc.dram_tensor("v", [Y, H], f32, kind="ExternalInput").ap()
    wq = nc.dram_tensor("wq", [P, H], f32, kind="ExternalInput").ap()
    out = nc.dram_tensor("out", [X, H], out_dt, kind="ExternalOutput").ap()

    q_t = q.rearrange("(n p) h -> n p h", p=P)
    v_t = v.rearrange("(n p) h -> n p h", p=P)
    out_r = out.rearrange("(t p) h -> t p h", p=P)

    with tile.TileContext(nc) as tc:
        with (
            tc.tile_pool(name="const", bufs=1) as constp,
            tc.tile_pool(name="qin", bufs=len(CHUNKS)) as qp,
            tc.tile_pool(name="vin", bufs=2 * len(CHUNKS)) as vp,
            tc.tile_pool(name="scr", bufs=3) as scr,
            tc.tile_pool(name="ebp", bufs=NT) as ebp,
            tc.tile_pool(name="small", bufs=1) as smallp,
            tc.tile_pool(name="ps_acc", bufs=1, space=bass.MemorySpace.PSUM) as psa,
        ):
            wq_b = constp.tile([P, H], bf16, tag="wq_b", name="wq_b")
            nc.gpsimd.dma_start(wq_b[:], wq)

            ones_col = constp.tile([P, 1], bf16, tag="ones_col", name="ones_col")
            nc.vector.memset(ones_col[:], 1.0)
            ones_big = constp.tile([P, P], bf16, tag="ones_big", name="ones_big")
            nc.vector.memset(ones_big[:], 1.0)

            sq_all = smallp.tile([P, NT], f32, tag="sq_all", name="sq_all")
            esq_all = smallp.tile([P, NT], f32, tag="esq_all", name="esq_all")

            ps_c0 = psa.tile([P, 512], f32, tag="ps_c0", name="ps_c0")
            ps_c1 = psa.tile([P, 512], f32, tag="ps_c1", name="ps_c1")
            ps_d = psa.tile([P, 1], f32, tag="ps_d", name="ps_d")

            starts = [sum(CHUNKS[:i]) for i in range(len(CHUNKS))]
            q_tiles = [qp.tile([P, cs * H], bf16, tag="q_sb",
                               name=f"q_sb{i}",
                               padded_shape=[P, max(CHUNKS) * H])
                       for i, cs in enumerate(CHUNKS)]
            # v half-column tiles: [P, cs*512] per (chunk, half)
            v_tiles = [[vp.tile([P, cs * 512], bf16, tag="v_bf",
                                name=f"v_bf{i}_{j}",
                                padded_shape=[P, max(CHUNKS) * 512])
                        for j in range(2)]
                       for i, cs in enumerate(CHUNKS)]

            # ---- DMA issue order: q stream, then v half 0, then v half 1
            for i, cs in enumerate(CHUNKS):
                s = starts[i]
                nc.gpsimd.dma_start(
                    q_tiles[i][:].rearrange("p (t h) -> p t h", t=cs),
                    q_t[s:s + cs].rearrange("n p h -> p n h"))
            for j in range(2):
                for i, cs in enumerate(CHUNKS):
                    s = starts[i]
                    src = v_t[s:s + cs, :, j * 512:(j + 1) * 512]
                    nc.gpsimd.dma_start(
                        v_tiles[i][j][:].rearrange("p (t h) -> p t h", t=cs),
                        src.rearrange("n p h -> p n h"))

            # ---- sq / esq / esq_b / d, paced with the q stream
            esq_bs = []
            yt = 0
            for ci, cs in enumerate(CHUNKS):
                q_sb = q_tiles[ci]
                sc = scr.tile([P, cs * H], bf16, tag="sc", name="sc",
                              padded_shape=[P, max(CHUNKS) * H])
                nc.vector.tensor_mul(
                    sc[:].rearrange("p (t h) -> p t h", t=cs),
                    q_sb[:].rearrange("p (t h) -> p t h", t=cs),
                    wq_b[:].unsqueeze(1).broadcast_to([P, cs, H]))
                for t in range(cs):
                    if yt % 2 == 1:
                        nc.vector.reduce_sum(
                            sq_all[:, yt:yt + 1], sc[:, t * H:(t + 1) * H],
                            axis=mybir.AxisListType.X)
                    else:
                        dump = scr.tile([P, H], bf16, tag="dump", name="dump")
                        nc.scalar.activation(
                            dump[:], sc[:, t * H:(t + 1) * H],
                            mybir.ActivationFunctionType.Copy,
                            accum_out=sq_all[:, yt:yt + 1])
                    nc.scalar.activation(
                        esq_all[:, yt:yt + 1], sq_all[:, yt:yt + 1],
                        mybir.ActivationFunctionType.Exp)
                    esq_b = ebp.tile([P, P], bf16, tag="esq_b",
                                     name=f"esq_b{yt}")
                    nc.vector.tensor_scalar_mul(
                        esq_b[:], ones_big[:], esq_all[:, yt:yt + 1])
                    esq_bs.append(esq_b)
                    nc.tensor.matmul(
                        ps_d[:], esq_b[:], ones_col[:],
                        start=(yt == 0), stop=(yt == NT - 1))
                    # c0 matmul interleaved here: PE consumes the vh0
                    # stream as it arrives instead of queuing all c0 work
                    # behind the last d-matmul (program-order FIFO)
                    nc.tensor.matmul(
                        ps_c0[:], esq_b[:],
                        v_tiles[ci][0][:, t * 512:(t + 1) * 512],
                        start=(yt == 0), stop=(yt == NT - 1))
                    yt += 1

            inv_d = smallp.tile([P, 1], f32, tag="inv_d", name="inv_d")
            nc.vector.reciprocal(inv_d[:], ps_d[:])

            bc_sb = smallp.tile([P, H], out_dt, tag="bc_sb", name="bc_sb")

            # ---- half 0: c0 already accumulated in the q-phase loop;
            # scale on ACT (idle here; its sequencer is not yet issuing)
            nc.scalar.activation(
                bc_sb[:, 0:512], ps_c0[:],
                mybir.ActivationFunctionType.Copy, scale=inv_d[:])
            for t in range(NT):
                eng = nc.sync if t % 2 == 0 else nc.scalar
                eng.dma_start(out_r[t, :, 0:512], bc_sb[:, 0:512])

            # ---- half 1: accumulate as vh1 streams, scale on DVE (the
            # Scalar sequencer is busy issuing h0 output DMAs by now)
            yt = 0
            for ci, cs in enumerate(CHUNKS):
                for t in range(cs):
                    nc.tensor.matmul(
                        ps_c1[:], esq_bs[yt],
                        v_tiles[ci][1][:, t * 512:(t + 1) * 512],
                        start=(yt == 0), stop=(yt == NT - 1))
                    yt += 1
            nc.vector.tensor_scalar_mul(bc_sb[:, 512:H], ps_c1[:], inv_d[:])
            for t in range(NT):
                eng = nc.sync if t % 2 == 0 else nc.scalar
                eng.dma_start(out_r[t, :, 512:H], bc_sb[:, 512:H])
    nc.compile()
    return nc


def _get_nc():
    if "nc" not in _cache:
        _cache["nc"] = _build()
    return _cache["nc"]


def _in_maps(q, k, v, W, b):
    q = np.asarray(q, dtype=np.float32)
    v = np.asarray(v, dtype=np.float32)
    W = np.asarray(W, dtype=np.float32)
    wq = np.ascontiguousarray(np.broadcast_to(W[H:], (P, H)))
    return [
        {"q": np.ascontiguousarray(q[c]),
         "v": np.ascontiguousarray(v[c]),
         "wq": wq}
        for c in range(N_CORES)
    ]


def kernel(q, k, v, W, b):
    from concourse.bass_utils import run_bass_kernel_spmd

    nc = _get_nc()
    res = run_bass_kernel_spmd(nc, _in_maps(q, k, v, W, b),
                               core_ids=list(range(N_CORES)))
    outs = [np.asarray(res.results[c]["out"]).astype(np.float32)
            for c in range(N_CORES)]
    return np.stack(outs)


# revision 25
# speedup vs baseline: 1.0985x; 1.0053x over previous
"""BiAttn kernel for 8 TRN2 NeuronCores.

The additive score e[b,x,y] = k[b,x]@Wk + q[b,y]@Wq + b is constant along
each softmax row up to the q-term, and softmax is shift-invariant, so the
attention weights are independent of x: out[b,x,:] = sum_y p[y] v[b,y,:]
with p = softmax(q_b @ Wq). k and the bias cancel; the whole [B,X,Y]
attention collapses to one weighted average per batch, broadcast over X.

Sharding: one batch per core (pure data parallel, no collectives).
Per core: read q_b,v_b (16MB f32, SWDGE DMAs casting to bf16 inline),
write out_b (4MB bf16, host upcasts). Rel err ~3e-3 vs the 2e-2 gate.

Structure (all phases stream; DMA never idles):
- q streams first; per tile: DVE mult by Wq (stride-0 broadcast AP),
  reduction alternating ACT activation(Copy, accum_out)/DVE reduce_sum,
  then ONE ACT op applies Exp to a stride-0 broadcast view of the sq
  column and writes the [128,128] replicated stationary tile esq_b.
- PE interleaves per tile: d += esq_b@ones, c0 += esq_b@vh0 — both land
  pre-broadcast on all 128 partitions (M=128 costs the same as M=1).
- v streams in column halves; when half 0 closes, ACT scales c0 by 1/d
  and its 2MB write overlaps the half-1 read; c1 accumulates behind the
  vh1 stream, DVE scales it, leaving only the last 2MB write serial.

Measured 66-78us/NEFF fleet-noise dependent (~14us fixed NEFF overhead).
fp32 matmuls would cost two LOW_HIGH passes - everything engine-side is
bf16 except sq scalars and PSUM accumulation."""

import sys

import numpy as np

for _p in ("/opt/trn_rl_repo",):
    if _p not in sys.path:
        sys.path.insert(0, _p)

B, X, Y, H = 8, 2048, 2048, 1024
N_CORES = 8
P = 128
NT = Y // P
CHUNKS = [2, 2, 2, 2, 2, 2, 2, 1, 1]
assert sum(CHUNKS) == NT
OUT_DTYPE = "bfloat16"

_cache = {}


def _build():
    import concourse.bass as bass
    import concourse.mybir as mybir
    from concourse import bacc, tile

    f32 = mybir.dt.float32
    bf16 = mybir.dt.bfloat16
    out_dt = getattr(mybir.dt, OUT_DTYPE)

    nc = bacc.Bacc("TRN2", target_bir_lowering=False, debug=False,
                   num_devices=N_CORES, name="biattn")

    q = nc.dram_tensor("q", [Y, H], f32, kind="ExternalInput").ap()
    v = nc.dram_tensor("v", [Y, H], f32, kind="ExternalInput").ap()
    wq = nc.dram_tensor("wq", [P, H], f32, kind="ExternalInput").ap()
    out = nc.dram_tensor("out", [X, H], out_dt, kind="ExternalOutput").ap()

    q_t = q.rearrange("(n p) h -> n p h", p=P)
    v_t = v.rearrange("(n p) h -> n p h", p=P)
    out_r = out.rearrange("(t p) h -> t p h", p=P)

    with tile.TileContext(nc) as tc:
        with (
            tc.tile_pool(name="const", bufs=1) as constp,
            tc.tile_pool(name="qin", bufs=len(CHUNKS)) as qp,
            tc.tile_pool(name="vin", bufs=2 * len(CHUNKS)) as vp,
            tc.tile_pool(name="scr", bufs=3) as scr,
            tc.tile_pool(name="ebp", bufs=NT) as ebp,
            tc.tile_pool(name="small", bufs=1) as smallp,
            tc.tile_pool(name="ps_acc", bufs=1, space=bass.MemorySpace.PSUM) as psa,
        ):
            wq_b = constp.tile([P, H], bf16, tag="wq_b", name="wq_b")
            nc.gpsimd.dma_start(wq_b[:], wq)

            ones_col = constp.tile([P, 1], bf16, tag="ones_col", name="ones_col")
            nc.vector.memset(ones_col[:], 1.0)

            sq_all = smallp.tile([P, NT], f32, tag="sq_all", name="sq_all")

            ps_c0 = psa.tile([P, 512], f32, tag="ps_c0", name="ps_c0")
            ps_c1 = psa.tile([P, 512], f32, tag="ps_c1", name="ps_c1")
            ps_d = psa.tile([P, 1], f32, tag="ps_d", name="ps_d")

            starts = [sum(CHUNKS[:i]) for i in range(len(CHUNKS))]
            q_tiles = [qp.tile([P, cs * H], bf16, tag="q_sb",
                               name=f"q_sb{i}",
                               padded_shape=[P, max(CHUNKS) * H])
                       for i, cs in enumerate(CHUNKS)]
            # v half-column tiles: [P, cs*512] per (chunk, half)
            v_tiles = [[vp.tile([P, cs * 512], bf16, tag="v_bf",
                                name=f"v_bf{i}_{j}",
                                padded_shape=[P, max(CHUNKS) * 512])
                        for j in range(2)]
                       for i, cs in enumerate(CHUNKS)]

            # ---- DMA issue order: q stream, then v half 0, then v half 1
            for i, cs in enumerate(CHUNKS):
                s = starts[i]
                nc.gpsimd.dma_start(
                    q_tiles[i][:].rearrange("p (t h) -> p t h", t=cs),
                    q_t[s:s + cs].rearrange("n p h -> p n h"))
            for j in range(2):
                for i, cs in enumerate(CHUNKS):
                    s = starts[i]
                    src = v_t[s:s + cs, :, j * 512:(j + 1) * 512]
                    nc.gpsimd.dma_start(
                        v_tiles[i][j][:].rearrange("p (t h) -> p t h", t=cs),
                        src.rearrange("n p h -> p n h"))

            # ---- sq / esq / esq_b / d, paced with the q stream
            esq_bs = []
            yt = 0
            for ci, cs in enumerate(CHUNKS):
                q_sb = q_tiles[ci]
                sc = scr.tile([P, cs * H], bf16, tag="sc", name="sc",
                              padded_shape=[P, max(CHUNKS) * H])
                nc.vector.tensor_mul(
                    sc[:].rearrange("p (t h) -> p t h", t=cs),
                    q_sb[:].rearrange("p (t h) -> p t h", t=cs),
                    wq_b[:].unsqueeze(1).broadcast_to([P, cs, H]))
                for t in range(cs):
                    if yt % 2 == 1:
                        nc.vector.reduce_sum(
                            sq_all[:, yt:yt + 1], sc[:, t * H:(t + 1) * H],
                            axis=mybir.AxisListType.X)
                    else:
                        dump = scr.tile([P, H], bf16, tag="dump", name="dump")
                        nc.scalar.activation(
                            dump[:], sc[:, t * H:(t + 1) * H],
                            mybir.ActivationFunctionType.Copy,
                            accum_out=sq_all[:, yt:yt + 1])
                    # fused exp+broadcast: ACT reads the sq column via a
                    # stride-0 AP and writes the replicated [128,128]
                    # stationary tile directly (no DVE hop, no esq_all)
                    esq_b = ebp.tile([P, P], bf16, tag="esq_b",
                                     name=f"esq_b{yt}")
                    nc.scalar.activation(
                        esq_b[:], sq_all[:, yt:yt + 1].broadcast_to([P, P]),
                        mybir.ActivationFunctionType.Exp)
                    esq_bs.append(esq_b)
                    nc.tensor.matmul(
                        ps_d[:], esq_b[:], ones_col[:],
                        start=(yt == 0), stop=(yt == NT - 1))
                    # c0 matmul interleaved here: PE consumes the vh0
                    # stream as it arrives instead of queuing all c0 work
                    # behind the last d-matmul (program-order FIFO)
                    nc.tensor.matmul(
                        ps_c0[:], esq_b[:],
                        v_tiles[ci][0][:, t * 512:(t + 1) * 512],
                        start=(yt == 0), stop=(yt == NT - 1))
                    yt += 1

            inv_d = smallp.tile([P, 1], f32, tag="inv_d", name="inv_d")
            nc.vector.reciprocal(inv_d[:], ps_d[:])

            bc_sb = smallp.tile([P, H], out_dt, tag="bc_sb", name="bc_sb")

            # ---- half 0: c0 already accumulated in the q-phase loop;
            # scale on ACT (idle here; its sequencer is not yet issuing)
            nc.scalar.activation(
                bc_sb[:, 0:512], ps_c0[:],
                mybir.ActivationFunctionType.Copy, scale=inv_d[:])
            for t in range(NT):
                eng = nc.sync if t % 2 == 0 else nc.scalar
                eng.dma_start(out_r[t, :, 0:512], bc_sb[:, 0:512])

            # ---- half 1: accumulate as vh1 streams, scale on DVE (the
            # Scalar sequencer is busy issuing h0 output DMAs by now)
            yt = 0
            for ci, cs in enumerate(CHUNKS):
                for t in range(cs):
                    nc.tensor.matmul(
                        ps_c1[:], esq_bs[yt],
                        v_tiles[ci][1][:, t * 512:(t + 1) * 512],
                        start=(yt == 0), stop=(yt == NT - 1))
                    yt += 1
            nc.vector.tensor_scalar_mul(bc_sb[:, 512:H], ps_c1[:], inv_d[:])
            for t in range(NT):
                eng = nc.sync if t % 2 == 0 else nc.scalar
                eng.dma_start(out_r[t, :, 512:H], bc_sb[:, 512:H])
    nc.compile()
    return nc


def _get_nc():
    if "nc" not in _cache:
        _cache["nc"] = _build()
    return _cache["nc"]


def _in_maps(q, k, v, W, b):
    q = np.asarray(q, dtype=np.float32)
    v = np.asarray(v, dtype=np.float32)
    W = np.asarray(W, dtype=np.float32)
    wq = np.ascontiguousarray(np.broadcast_to(W[H:], (P, H)))
    return [
        {"q": np.ascontiguousarray(q[c]),
         "v": np.ascontiguousarray(v[c]),
         "wq": wq}
        for c in range(N_CORES)
    ]


def kernel(q, k, v, W, b):
    from concourse.bass_utils import run_bass_kernel_spmd

    nc = _get_nc()
    res = run_bass_kernel_spmd(nc, _in_maps(q, k, v, W, b),
                               core_ids=list(range(N_CORES)))
    outs = [np.asarray(res.results[c]["out"]).astype(np.float32)
            for c in range(N_CORES)]
    return np.stack(outs)
